# revision 2
# baseline (speedup 1.0000x reference)
"""BiAttention kernel for Trainium2, 8 NeuronCores, data-parallel over batch.

Math (per batch element, matching the reference):
    S[i,j]  = c[i]@w_c + q[j]@w_q + (c[i]*w_m)@q[j]       # [c_len, q_len]
    c2q     = softmax_j(S) @ q                            # [c_len, D]
    b       = softmax_i(max_j S[i,j])                     # [c_len]
    q2c     = b @ c                                       # [D]
    out     = [c, c2q, c*c2q, c*q2c[None,:]]              # [c_len, 4D]

Device algorithm (per core, one batch element), v2:
  * Transposed score layout T = S^T - cwc (q on partitions, c on free):
    E = exp(T + qwq) via ACT with per-partition bias; cwc cancels in
    softmax_j.  No max subtraction (|S| <= ~8, exp fits fp16/f32).
  * softmax_j(S) @ q == (E^T @ [q|1]) / l with l from the ones-column.
  * max_j S path: max_j exp = exp(max_j), row max on E (DVE max tree +
    PE transpose + free reduce); softmax-i weights wv = maxE * exp(cwc).
  * q2c via PE: per c-tile matvec q2c_half += c_tile.T @ wv_col into a
    persistent PSUM bank; denominator via s=rowsum(wv), den = s.T @ 1.
    Broadcast with a K=3 ones matmul: rhs rows [q2c_h0|0|0],[0|q2c_h1|0],
    [0|0|den,den] -> [128, 258] all-partition numerator+den.  No GPSIMD
    partition_all_reduce, no serial accumulation chain.
  * ACT engine runs ONLY Exp (avoids activation-table reloads); all
    copies/muls go to DVE/Pool.
  * Output blocks 0..2 are assembled per chunk in SBUF ([c|b2|b3] rows,
    3KB contiguous per row) and written with ONE DMA per 1024-row chunk;
    block 3 (c * q2c) trails after the global reduction.

Inputs are sharded on the host: core i gets q[i], c[i], w.  No collectives.
"""
import numpy as np

import concourse.bacc as bacc
import concourse.mybir as mybir
from concourse import bass_isa, tile
from concourse.bass_utils import run_bass_kernel_spmd
from concourse.masks import make_identity

B = 8
QL = 512          # q_len
CL = 4096         # c_len
D = 256           # feature dim
ODIM = 4 * D      # output feature dim
P = 128           # partitions
NQT = QL // P     # 4   q tiles
NKT = D // P      # 2   contraction tiles
NT = CL // P      # 32  c tiles


def set_chunks(n):
    global NCHUNK, CHUNK, TPC, NH, HC
    NCHUNK = n                 # c chunks per core
    CHUNK = CL // NCHUNK       # rows per chunk
    TPC = CHUNK // P           # c tiles per chunk
    NH = max(1, CHUNK // 512)  # score-matmul halves (moving-N <= 512)
    HC = CHUNK // NH


set_chunks(4)
ACT_HELP = False  # ScalarE runs only Exp (act-table reloads on HW)
POOL_OK = False   # keep elementwise off GpSimd (per-op dispatch cost on HW)

F32 = mybir.dt.float32
FP16 = mybir.dt.float16
EXP = mybir.ActivationFunctionType.Exp
MAX = mybir.AluOpType.max
MULT = mybir.AluOpType.mult
ADD = mybir.AluOpType.add
AXX = mybir.AxisListType.X


def _emit(nc, tc, reps=1):
    q = nc.dram_tensor("q", [QL, D], F32, kind="ExternalInput").ap()
    c = nc.dram_tensor("c", [CL, D], F32, kind="ExternalInput").ap()
    w = nc.dram_tensor("w", [3 * D], F32, kind="ExternalInput").ap()
    out = nc.dram_tensor("out", [CL, ODIM], F32, kind="ExternalOutput").ap()
    for _ in range(reps):
        _emit_body(nc, tc, q, c, w, out)


def _emit_body(nc, tc, q, c, w, out):
    from contextlib import ExitStack
    stack = ExitStack()
    cst = stack.enter_context(tc.tile_pool(name="cst", bufs=1))
    per = stack.enter_context(tc.tile_pool(name="per", bufs=1))
    wrk = stack.enter_context(tc.tile_pool(name="wrk", bufs=2))
    epl = stack.enter_context(tc.tile_pool(name="epl", bufs=2))
    c16p = stack.enter_context(tc.tile_pool(name="c16p", bufs=2))
    ost = stack.enter_context(tc.tile_pool(name="ost", bufs=2))
    o4p = stack.enter_context(tc.tile_pool(name="o4p", bufs=2))
    ps_tp = stack.enter_context(tc.tile_pool(name="ps_tp", bufs=2, space="PSUM"))
    ps_tm = stack.enter_context(tc.tile_pool(name="ps_tm", bufs=1, space="PSUM"))
    ps_st = stack.enter_context(tc.tile_pool(name="ps_st", bufs=2, space="PSUM"))
    ps_at = stack.enter_context(tc.tile_pool(name="ps_at", bufs=2, space="PSUM"))
    ps_qc = stack.enter_context(tc.tile_pool(name="ps_qc", bufs=1, space="PSUM"))

    # ---------------- constants ----------------
    ident = cst.tile([P, P], F32)
    make_identity(nc, ident[:])
    ident_hf = cst.tile([P, P], FP16)
    make_identity(nc, ident_hf[:])

    w_f32 = cst.tile([P, 6], F32)   # col k = w[k*128:(k+1)*128]
    nc.scalar.dma_start(out=w_f32[:], in_=w.rearrange("(k p) -> p k", p=P))
    q_sb = per.tile([P, NQT * D], F32)          # q, natural layout
    nc.sync.dma_start(out=q_sb[:].rearrange("p (a d) -> p a d", a=NQT),
                      in_=q.rearrange("(a p) d -> p a d", p=P))
    # [w_q_k | w_c_k] pairs per k-tile for the tiny per-tile matmuls
    w_r = cst.tile([P, 4], FP16)
    for j, col in enumerate((0, 2, 1, 3)):   # wq_h0, wc_h0, wq_h1, wc_h1
        nc.vector.tensor_copy(w_r[:, j:j + 1], w_f32[:, col:col + 1])
    ones_t = cst.tile([P, 2], F32)
    nc.vector.memset(ones_t[:], 1.0)
    ones_m = cst.tile([P, P], F32)
    nc.vector.memset(ones_m[:], 1.0)

    # ---------------- persistent buffers ----------------
    qa = per.tile([P, NQT * 258], FP16)         # [q | 1 | pad] attention rhs
    qmT = per.tile([P, NKT * QL], FP16)         # (w_m (.) q)^T, [d, q]
    qTr = per.tile([P, NKT * QL], FP16)         # raw q^T for qwq
    qwq = per.tile([P, NQT], F32)               # q @ w_q, per q-tile column
    c_sb = per.tile([P, NT * D], F32)           # c, natural layout, all tiles
    cT = per.tile([P, NKT * CL], FP16)          # c^T, [d, c]
    ewc = per.tile([P, NT], F32)                # exp(c @ w_c) per c-tile column
    wv = per.tile([P, NT], F32)                 # softmax-i weights per c-tile
    wv16 = per.tile([P, NT], FP16)              # fp16 wv (q2c matvec rhs)
    ssum = per.tile([P, 1], F32)                # rowsum of wv
    bc_sb = per.tile([P, 258], F32)             # broadcast matmul rhs (row 0)
    q2cn = per.tile([P, D], F32)                # broadcast q2c numerator
    inv_den = per.tile([P, 1], F32)

    # ---------------- q setup: transpose, qwq, q_aug ----------------
    for a in range(NQT):
        nc.vector.tensor_copy(qa[:, a * 258:a * 258 + 256], q_sb[:, a * D:(a + 1) * D])
        nc.vector.tensor_copy(qa[:, a * 258 + 256:a * 258 + 258], ones_t[:])
    for k in range(NKT):
        tp = ps_tp.tile([P, 512], FP16, tag="tp")
        for a in range(NQT):
            nc.tensor.transpose(tp[:, a * P:(a + 1) * P],
                                qa[:, a * 258 + k * P:a * 258 + (k + 1) * P],
                                ident_hf[:])
        nc.vector.tensor_scalar_mul(qmT[:, k * QL:(k + 1) * QL], tp[:],
                                    w_f32[:, 4 + k:5 + k])
        nc.vector.tensor_copy(qTr[:, k * QL:(k + 1) * QL], tp[:])
    pwq = ps_tp.tile([P, HC], F32, tag="tp")
    for a in range(NQT):
        for k in range(NKT):
            nc.tensor.matmul(pwq[:, 2 * a:2 * a + 2],
                             qTr[:, k * QL + a * P:k * QL + (a + 1) * P],
                             w_r[:, 2 * k:2 * k + 2], start=(k == 0), stop=(k == NKT - 1))
    nc.vector.tensor_copy(qwq[:].rearrange("p (a o) -> p a o", o=1),
                          pwq[:, 0:2 * NQT].rearrange("p (a s) -> p a s", s=2)[:, :, 0:1])

    # ---------------- main pass over c chunks ----------------
    # q2c row accumulator: [0:256] = sum_i wv_i * c[i,:], [256:258] = den
    q2a = ps_qc.tile([P, 258], F32, tag="q2cp")
    for ci in range(NCHUNK):
        c0 = ci * CHUNK
        t0 = ci * TPC
        nc.scalar.dma_start(
            out=c_sb[:, t0 * D:(t0 + TPC) * D].rearrange("p (t d) -> p t d", t=TPC),
            in_=c[c0:c0 + CHUNK, :].rearrange("(t p) d -> p t d", p=P))
        # fp16 copy of the chunk (transpose input + q2c matvec stationary)
        c16 = c16p.tile([P, TPC * D], FP16, tag="c16")
        nc.vector.tensor_copy(c16[:], c_sb[:, t0 * D:(t0 + TPC) * D])
        # c^T tiles: per (k, half) fp16 transposes into one psum bank + copy
        for k in range(NKT):
            for h in range(NH):
                tp = ps_tp.tile([P, HC], FP16, tag="tp")
                for j in range(HC // P):
                    t = h * (HC // P) + j
                    nc.tensor.transpose(tp[:, j * P:(j + 1) * P],
                                        c16[:, t * D + k * P:t * D + (k + 1) * P],
                                        ident_hf[:])
                if ACT_HELP and k == 1:
                    nc.scalar.copy(
                        cT[:, k * CL + c0 + h * HC:k * CL + c0 + (h + 1) * HC], tp[:])
                else:
                    nc.vector.tensor_copy(
                        cT[:, k * CL + c0 + h * HC:k * CL + c0 + (h + 1) * HC], tp[:])
        # exp(c @ w_c): 16 tiny matmuls into one [128,16] psum, one strided exp
        pw = ps_tp.tile([P, HC], F32, tag="tp")
        for tt in range(TPC):
            t = t0 + tt
            for k in range(NKT):
                nc.tensor.matmul(pw[:, 2 * tt:2 * tt + 2],
                                 cT[:, k * CL + t * P:k * CL + (t + 1) * P],
                                 w_r[:, 2 * k:2 * k + 2], start=(k == 0), stop=(k == NKT - 1))
        nc.scalar.activation(
            ewc[:, t0:t0 + TPC].rearrange("p (t o) -> p t o", o=1),
            pw[:, 0:2 * TPC].rearrange("p (t s) -> p t s", s=2)[:, :, 1:2], EXP)
        # scores E = exp(T + qwq), [q, c] layout, fp16, h-major so the
        # first half's attention can start after 4 exps; per-half max path
        E = epl.tile([P, NQT * CHUNK], FP16, tag="E")
        mx = wrk.tile([P, TPC], F32, tag="mx")
        hp = TPC // NH
        for h in range(NH):
            for a in range(NQT):
                st = ps_st.tile([P, HC], F32, tag="st")
                for k in range(NKT):
                    nc.tensor.matmul(st[:], qmT[:, k * QL + a * P:k * QL + (a + 1) * P],
                                     cT[:, k * CL + c0 + h * HC:k * CL + c0 + (h + 1) * HC],
                                     start=(k == 0), stop=(k == NKT - 1))
                nc.scalar.activation(E[:, a * CHUNK + h * HC:a * CHUNK + (h + 1) * HC],
                                     st[:], EXP, bias=qwq[:, a:a + 1])
            # row-max over the 4 q-tiles for this half, partition-reduce via PE
            m01 = wrk.tile([P, HC], FP16, tag="m01")
            m23 = wrk.tile([P, HC], FP16, tag="m23")
            m_1 = wrk.tile([P, HC], FP16, tag="m_1")
            nc.vector.tensor_tensor(m01[:], E[:, 0 * CHUNK + h * HC:0 * CHUNK + (h + 1) * HC],
                                    E[:, 1 * CHUNK + h * HC:1 * CHUNK + (h + 1) * HC], MAX)
            nc.vector.tensor_tensor(m23[:], E[:, 2 * CHUNK + h * HC:2 * CHUNK + (h + 1) * HC],
                                    E[:, 3 * CHUNK + h * HC:3 * CHUNK + (h + 1) * HC], MAX)
            nc.vector.tensor_tensor(m_1[:], m01[:], m23[:], MAX)
            tpm = ps_tm.tile([P, hp * P], FP16, tag="tm")
            for j in range(hp):
                nc.tensor.transpose(tpm[:, j * P:(j + 1) * P],
                                    m_1[:, j * P:(j + 1) * P], ident_hf[:])
            nc.vector.reduce_max(mx[:, h * hp:(h + 1) * hp],
                                 tpm[:].rearrange("p (t x) -> p t x", t=hp),
                                 axis=AXX)
            nc.vector.tensor_tensor(wv[:, t0 + h * hp:t0 + (h + 1) * hp],
                                    mx[:, h * hp:(h + 1) * hp],
                                    ewc[:, t0 + h * hp:t0 + (h + 1) * hp], MULT)
            nc.vector.tensor_copy(wv16[:, t0 + h * hp:t0 + (h + 1) * hp],
                                  wv[:, t0 + h * hp:t0 + (h + 1) * hp])
            # q2c numerator row: += wv16_col.T @ c16_tile  -> [1, 256]
            for j in range(hp):
                tt = h * hp + j
                t = t0 + tt
                nc.tensor.matmul(q2a[0:1, 0:D], wv16[:, t:t + 1],
                                 c16[:, tt * D:(tt + 1) * D],
                                 start=(t == 0), stop=(t == NT - 1))
        # attention + output blocks 0..2 for this chunk
        o3 = ost.tile([P, TPC * 3 * D], F32, tag="o3")
        for tp2 in range(TPC // 2):
            cpeng = nc.gpsimd if POOL_OK else (nc.scalar, nc.vector)[tp2 % 2]
            if cpeng is nc.scalar:
                cpeng.copy(
                    o3[:, 2 * tp2 * 3 * D:2 * (tp2 + 1) * 3 * D].rearrange(
                        "p (t x) -> p t x", t=2)[:, :, 0:D],
                    c_sb[:, (t0 + 2 * tp2) * D:(t0 + 2 * (tp2 + 1)) * D].rearrange(
                        "p (t d) -> p t d", t=2))
            else:
                cpeng.tensor_copy(
                    o3[:, 2 * tp2 * 3 * D:2 * (tp2 + 1) * 3 * D].rearrange(
                        "p (t x) -> p t x", t=2)[:, :, 0:D],
                    c_sb[:, (t0 + 2 * tp2) * D:(t0 + 2 * (tp2 + 1)) * D].rearrange(
                        "p (t d) -> p t d", t=2))
            for s in range(2):
                tt = 2 * tp2 + s
                t = t0 + tt
                po = ps_at.tile([P, 258], F32, tag="at")
                for a in range(NQT):
                    nc.tensor.matmul(
                        po[:],
                        E[:, a * CHUNK + tt * P:a * CHUNK + (tt + 1) * P],
                        qa[:, a * 258:(a + 1) * 258],
                        start=(a == 0), stop=(a == NQT - 1))
                invl = wrk.tile([P, 1], F32, tag="invl")
                nc.vector.reciprocal(invl[:], po[:, 256:257])
                b2 = o3[:, tt * 3 * D + D:tt * 3 * D + 2 * D]
                b3 = o3[:, tt * 3 * D + 2 * D:tt * 3 * D + 3 * D]
                if ACT_HELP and s == 1:
                    nc.scalar.mul(b2, po[:, 0:D], invl[:])
                else:
                    nc.vector.tensor_scalar_mul(b2, po[:, 0:D], invl[:])
                if POOL_OK:
                    b3eng = (nc.gpsimd, nc.gpsimd, nc.gpsimd, nc.vector)[tp2 % 4]
                else:
                    b3eng = nc.vector
                b3eng.tensor_tensor(b3, b2, c_sb[:, t * D:(t + 1) * D], MULT)
            nc.sync.dma_start(
                out=out[c0 + 2 * tp2 * P:c0 + 2 * (tp2 + 1) * P, 0:3 * D].rearrange(
                    "(t p) d -> p t d", p=P),
                in_=o3[:, 2 * tp2 * 3 * D:2 * (tp2 + 1) * 3 * D].rearrange(
                    "p (t x) -> p t x", t=2))

    # ---------------- q2c finalize (all on PE/DVE) + block 3 ----------------
    nc.vector.reduce_sum(ssum[:], wv[:], axis=AXX)
    nc.tensor.matmul(q2a[0:1, 256:258], ssum[:], ones_t[:], start=True, stop=True)
    nc.vector.tensor_copy(bc_sb[0:1, :], q2a[0:1, :])
    bps = ps_qc.tile([P, 258], F32, tag="q2cp")
    nc.tensor.matmul(bps[:], ones_m[0:1, :], bc_sb[0:1, :], start=True, stop=True)
    nc.vector.reciprocal(inv_den[:], bps[:, 256:257])
    nc.vector.tensor_scalar_mul(q2cn[:], bps[:, 0:D], inv_den[:])
    hq = TPC // 2
    for ci in range(NCHUNK):
        c0 = ci * CHUNK
        t0 = ci * TPC
        o4 = o4p.tile([P, TPC * D], F32, tag="o4")
        for h in range(2):
            for j in range(hq):
                tt = h * hq + j
                t = t0 + tt
                if POOL_OK:
                    o4eng = (nc.vector, nc.gpsimd, nc.vector, nc.gpsimd,
                             nc.vector, nc.gpsimd, nc.vector, nc.gpsimd)[tt % 8]
                    o4eng.tensor_tensor(o4[:, tt * D:(tt + 1) * D],
                                        c_sb[:, t * D:(t + 1) * D], q2cn[:], MULT)
                else:
                    nc.vector.tensor_tensor(o4[:, tt * D:(tt + 1) * D],
                                            c_sb[:, t * D:(t + 1) * D], q2cn[:], MULT)
            nc.sync.dma_start(
                out=out[c0 + h * hq * P:c0 + (h + 1) * hq * P,
                        3 * D:4 * D].rearrange("(t p) d -> p t d", p=P),
                in_=o4[:, h * hq * D:(h + 1) * hq * D].rearrange(
                    "p (t d) -> p t d", t=hq))

    stack.close()


def build(reps=1, loop=0):
    nc = bacc.Bacc("TRN2", target_bir_lowering=False, debug=False)
    with tile.TileContext(nc) as tc:
        if loop:
            q = nc.dram_tensor("q", [QL, D], F32, kind="ExternalInput").ap()
            c = nc.dram_tensor("c", [CL, D], F32, kind="ExternalInput").ap()
            w = nc.dram_tensor("w", [3 * D], F32, kind="ExternalInput").ap()
            out = nc.dram_tensor("out", [CL, ODIM], F32, kind="ExternalOutput").ap()
            with tc.For_i(0, loop, 1):
                _emit_body(nc, tc, q, c, w, out)
        else:
            _emit(nc, tc, reps=reps)
    nc.compile()
    return nc


_NC = None


def _run(q, c, w, **spmd_kwargs):
    global _NC
    if _NC is None:
        _NC = build()
    q = np.ascontiguousarray(np.asarray(q, dtype=np.float32))
    c = np.ascontiguousarray(np.asarray(c, dtype=np.float32))
    w = np.ascontiguousarray(np.asarray(w, dtype=np.float32))
    in_maps = [{"q": q[i], "c": c[i], "w": w} for i in range(B)]
    res = run_bass_kernel_spmd(_NC, in_maps, list(range(B)), **spmd_kwargs)
    out = np.stack([res.results[i]["out"] for i in range(B)])
    return out, res


def kernel(q, c, w):
    out, _ = _run(q, c, w)
    return out
